# revision 1
# baseline (speedup 1.0000x reference)
"""Multi-head attention (B=4, L=2048, D=512, H=8) on 8 Trainium2 cores.

Sharding: core c handles batch b = c//2, query rows [(c%2)*1024, +1024).
K/V projections are split across the two cores sharing a batch (each
projects its own 1024-token half) and exchanged with a pairwise
AllGather through shared DRAM, so attention is fully local afterward.

Device layouts (per core):
  xqT/xkT/xvT (512, 1024)  input slices, transposed (dmodel on partitions)
  qT_all / kT_all          projections kept transposed: head h lives in
                           dmodel-chunk tile h//2 at partition offset 64*(h%2)
  V_sb (128, 520) x16      V natural layout per kv chunk; head h at cols
                           [65h, 65h+64), col 65h+64 = ones (softmax denom)
  scoresT (128kv, 1024q)   PSUM; exp+mask+scale fused into one ACT op
  xsT_ext (65, 512)        PSUM, row 64 = softmax denominator
"""
import numpy as np
import ml_dtypes

import concourse.bacc as bacc
import concourse.bass as bass
import concourse.mybir as mybir
import concourse.tile as tile
from concourse.bass_utils import run_bass_kernel_spmd

F32 = mybir.dt.float32
BF16 = mybir.dt.bfloat16
AF = mybir.ActivationFunctionType

B, L, D = 4, 2048, 512
H, DK = 8, 64
N_CORES = 8
LQ = L // 2            # query rows per core / kv rows projected per core
P = 128
KVC = L // P           # 16 kv chunks
QT = LQ // P           # 8 query tiles of 128
MC = D // P            # 4 dmodel chunks
MASK_BIAS = np.float32(-1e30)

MM_DT = BF16
MM_NP = ml_dtypes.bfloat16 if MM_DT == BF16 else np.float32

_cache = {}


def _build():
    nc = bacc.Bacc("TRN2", target_bir_lowering=False, debug=False,
                   num_devices=N_CORES)

    xqT_d = nc.dram_tensor("xqT", [D, LQ], MM_DT, kind="ExternalInput").ap()
    xkT_d = nc.dram_tensor("xkT", [D, LQ], MM_DT, kind="ExternalInput").ap()
    xvT_d = nc.dram_tensor("xvT", [D, LQ], MM_DT, kind="ExternalInput").ap()
    wq_d = nc.dram_tensor("wq", [D, D], MM_DT, kind="ExternalInput").ap()
    wk_d = nc.dram_tensor("wk", [D, D], MM_DT, kind="ExternalInput").ap()
    wv_d = nc.dram_tensor("wv", [D, D], MM_DT, kind="ExternalInput").ap()
    wo_d = nc.dram_tensor("wo", [D, D], MM_DT, kind="ExternalInput").ap()
    bq_d = nc.dram_tensor("bq", [P, MC], F32, kind="ExternalInput").ap()
    bk_d = nc.dram_tensor("bk", [P, MC], F32, kind="ExternalInput").ap()
    bv_d = nc.dram_tensor("bv", [1, D], MM_DT, kind="ExternalInput").ap()
    bo_d = nc.dram_tensor("bo", [1, D], F32, kind="ExternalInput").ap()
    mb_d = nc.dram_tensor("mb", [P, KVC], F32, kind="ExternalInput").ap()
    out_d = nc.dram_tensor("out", [LQ, D], F32, kind="ExternalOutput").ap()

    PAIRS = [[2 * i, 2 * i + 1] for i in range(N_CORES // 2)]

    with tile.TileContext(nc) as tc:
        with tc.tile_pool(name="const", bufs=1) as cpool, \
             tc.tile_pool(name="xin", bufs=1) as xpool, \
             tc.tile_pool(name="proj", bufs=1) as prpool, \
             tc.tile_pool(name="attn", bufs=17) as apool, \
             tc.tile_pool(name="norm", bufs=4) as npool, \
             tc.tile_pool(name="outp", bufs=3) as opool, \
             tc.tile_pool(name="dram", bufs=1, space="DRAM") as dpool, \
             tc.tile_pool(name="ps", bufs=2, space="PSUM") as ps:

            def load_chunks(pool, ap2d, nm):
                out = []
                for kc in range(MC):
                    t = pool.tile([P, ap2d.shape[1]], ap2d.dtype,
                                  tag=f"{nm}{kc}", name=f"{nm}{kc}")
                    nc.sync.dma_start(t[:], ap2d[kc * P:(kc + 1) * P, :])
                    out.append(t)
                return out

            # interleave weight/input chunk loads in first-use order
            wq = load_chunks(cpool, wq_d, "wq")
            xqT = load_chunks(xpool, xqT_d, "xq")
            bq = cpool.tile_from(bq_d)
            wk = load_chunks(cpool, wk_d, "wk")
            xkT = load_chunks(xpool, xkT_d, "xk")
            bk = cpool.tile_from(bk_d)
            wv = load_chunks(cpool, wv_d, "wv")
            xvT = load_chunks(xpool, xvT_d, "xv")
            wo = load_chunks(cpool, wo_d, "wo")
            bv = cpool.tile_from(bv_d)
            bo = cpool.tile_from(bo_d)
            mb = cpool.tile_from(mb_d)
            ones1 = cpool.tile([1, P], MM_DT)
            nc.vector.memset(ones1[:], 1.0)
            bo_bc = cpool.tile([P, D], F32)
            nc.gpsimd.partition_broadcast(bo_bc[:], bo[:])

            # collective exchange buffers (pairwise AllGather of K/V halves)
            k_own_d = dpool.tile([MC, P, LQ], MM_DT)
            v_own_d = dpool.tile([KVC // 2, P, H * 65], MM_DT)
            k_all_d = dpool.tile([2, MC, P, LQ], MM_DT)
            v_all_d = dpool.tile([2, KVC // 2, P, H * 65], MM_DT)

            # ---- Q projection + own-half K projection (transposed) ----
            qT = [prpool.tile([P, LQ], MM_DT, tag=f"qT{m}", name=f"qT{m}")
                  for m in range(MC)]
            kTo = [prpool.tile([P, LQ], MM_DT, tag=f"kTo{m}", name=f"kTo{m}")
                   for m in range(MC)]
            for m in range(MC):
                for s in range(LQ // 512):
                    pp = ps.tile([P, 512], F32, tag="proj")
                    for kc in range(MC):
                        nc.tensor.matmul(
                            pp[:], wq[kc][:, m * P:(m + 1) * P],
                            xqT[kc][:, s * 512:(s + 1) * 512],
                            start=kc == 0, stop=kc == MC - 1)
                    nc.vector.tensor_scalar_add(qT[m][:, s * 512:(s + 1) * 512],
                                                pp[:], bq[:, m:m + 1])
                for s in range(LQ // 512):
                    pp = ps.tile([P, 512], F32, tag="proj")
                    for kc in range(MC):
                        nc.tensor.matmul(
                            pp[:], wk[kc][:, m * P:(m + 1) * P],
                            xkT[kc][:, s * 512:(s + 1) * 512],
                            start=kc == 0, stop=kc == MC - 1)
                    nc.vector.tensor_scalar_add(kTo[m][:, s * 512:(s + 1) * 512],
                                                pp[:], bk[:, m:m + 1])
                nc.sync.dma_start(k_own_d[m], kTo[m][:])

            # ---- own-half V projection (natural layout + ones columns) ----
            Vo = [prpool.tile([P, H * 65], MM_DT, tag=f"Vo{t}", name=f"Vo{t}")
                  for t in range(KVC // 2)]
            for t in range(KVC // 2):
                pv = ps.tile([P, D], F32, tag="proj")
                for kc in range(MC):
                    nc.tensor.matmul(pv[:], xvT[kc][:, t * P:(t + 1) * P],
                                     wv[kc][:, :], start=kc == 0, stop=False)
                nc.tensor.matmul(pv[:], ones1[0:1, :], bv[0:1, :],
                                 start=False, stop=True)
                vv = Vo[t].rearrange("p (g d) -> p g d", d=65)
                nc.vector.tensor_copy(vv[:, :, 0:64],
                                      pv.rearrange("p (g d) -> p g d", d=64))
                nc.vector.memset(vv[:, :, 64:65], 1.0)
                nc.sync.dma_start(v_own_d[t], Vo[t][:])

            # ---- pairwise K/V exchange ----
            nc.gpsimd.collective_compute(
                "AllGather", mybir.AluOpType.bypass, replica_groups=PAIRS,
                ins=[k_own_d[:]], outs=[k_all_d[:]])
            nc.gpsimd.collective_compute(
                "AllGather", mybir.AluOpType.bypass, replica_groups=PAIRS,
                ins=[v_own_d[:]], outs=[v_all_d[:]])

            kT = [prpool.tile([P, L], MM_DT, tag=f"kT{m}", name=f"kT{m}")
                  for m in range(MC)]
            for m in range(MC):
                for hf in range(2):
                    nc.sync.dma_start(kT[m][:, hf * LQ:(hf + 1) * LQ],
                                      k_all_d[hf, m])
            V = [prpool.tile([P, H * 65], MM_DT, tag=f"V{t}", name=f"V{t}")
                 for t in range(KVC)]
            for t in range(KVC):
                nc.sync.dma_start(V[t][:], v_all_d[t // (KVC // 2),
                                                   t % (KVC // 2)])

            # ---- flash attention per head ----
            xsT2 = [prpool.tile([P, LQ], MM_DT, tag=f"xs{hp}", name=f"xsT2_{hp}")
                    for hp in range(MC)]
            for h in range(H):
                hp, po = h // 2, 64 * (h % 2)
                at = []
                for c in range(KVC):
                    ss = ps.tile([P, 1024], F32, tag="scores", bufs=3)
                    for qh in range(2):
                        nc.tensor.matmul(
                            ss[:, qh * 512:(qh + 1) * 512],
                            kT[hp][po:po + 64, c * P:(c + 1) * P],
                            qT[hp][po:po + 64, qh * 512:(qh + 1) * 512],
                            start=True, stop=True)
                    a = apool.tile([P, 1024], MM_DT, tag="at")
                    nc.scalar.activation(a[:], ss[:], AF.Exp,
                                         bias=mb[:, c:c + 1], scale=0.125)
                    at.append(a)
                xs = [ps.tile([65, 512], F32, tag="proj", name=f"xs_h{h}_{qh}")
                      for qh in range(2)]
                for c in range(KVC):
                    for qh in range(2):
                        nc.tensor.matmul(
                            xs[qh][:], V[c][:, 65 * h:65 * h + 65],
                            at[c][:, qh * 512:(qh + 1) * 512],
                            start=c == 0, stop=c == KVC - 1)
                for qh in range(2):
                    srow = npool.tile([1, 512], F32, tag="srow")
                    nc.vector.tensor_copy(srow[:], xs[qh][64:65, :])
                    rec = npool.tile([1, 512], F32, tag="rec")
                    nc.vector.reciprocal_approx_fast(rec[:], srow[:])
                    bc = npool.tile([64, 512], F32, tag="bc")
                    nc.gpsimd.partition_broadcast(bc[:], rec[:])
                    nc.vector.tensor_mul(
                        xsT2[hp][po:po + 64, qh * 512:(qh + 1) * 512],
                        xs[qh][0:64, :], bc[:])

            # ---- output projection ----
            for qt in range(QT):
                po_ = ps.tile([P, D], F32, tag="proj")
                for hp in range(MC):
                    nc.tensor.matmul(po_[:], xsT2[hp][:, qt * P:(qt + 1) * P],
                                     wo[hp][:, :], start=hp == 0, stop=hp == MC - 1)
                osb = opool.tile([P, D], F32, tag="osb")
                nc.vector.tensor_add(osb[:], po_[:], bo_bc[:])
                nc.sync.dma_start(out_d[qt * P:(qt + 1) * P, :], osb[:])

    nc.compile()
    return nc


def _host_inputs(query, key, value, mask, Wq, bq, Wk, bk, Wv, bv, Wo, bo):
    """Build the 8 per-core input maps (all rank-dependence lives here)."""
    f32 = np.float32
    wq_ = np.ascontiguousarray(Wq).astype(MM_NP)
    wk_ = np.ascontiguousarray(Wk).astype(MM_NP)
    wv_ = np.ascontiguousarray(Wv).astype(MM_NP)
    wo_ = np.ascontiguousarray(Wo).astype(MM_NP)
    bq_ = np.ascontiguousarray(bq.astype(f32).reshape(MC, P).T)
    bk_ = np.ascontiguousarray(bk.astype(f32).reshape(MC, P).T)
    bv_ = bv.astype(MM_NP).reshape(1, D)
    bo_ = bo.astype(f32).reshape(1, D)
    in_maps = []
    for c in range(N_CORES):
        b, half = c // 2, c % 2
        sl = slice(half * LQ, (half + 1) * LQ)
        xqT = np.ascontiguousarray(query[b, sl, :].T).astype(MM_NP)
        xkT = np.ascontiguousarray(key[b, sl, :].T).astype(MM_NP)
        xvT = np.ascontiguousarray(value[b, sl, :].T).astype(MM_NP)
        mbias = np.where(mask[b] == 0, MASK_BIAS, f32(0.0)).astype(f32)
        mb_ = np.ascontiguousarray(mbias.reshape(KVC, P).T)
        in_maps.append({
            "xqT": xqT, "xkT": xkT, "xvT": xvT,
            "wq": wq_, "wk": wk_, "wv": wv_, "wo": wo_,
            "bq": bq_, "bk": bk_, "bv": bv_, "bo": bo_, "mb": mb_,
        })
    return in_maps


def kernel(query, key, value, mask, Wq, bq, Wk, bk, Wv, bv, Wo, bo):
    if "nc" not in _cache:
        _cache["nc"] = _build()
    nc = _cache["nc"]
    in_maps = _host_inputs(query, key, value, mask,
                           Wq, bq, Wk, bk, Wv, bv, Wo, bo)
    res = run_bass_kernel_spmd(nc, in_maps, list(range(N_CORES))).results
    out = np.empty((B, L, D), np.float32)
    for c in range(N_CORES):
        b, half = c // 2, c % 2
        out[b, half * LQ:(half + 1) * LQ, :] = res[c]["out"]
    return out



# revision 2
# speedup vs baseline: 1.9305x; 1.9305x over previous
"""Multi-head attention (B=4, L=2048, D=512, H=8) on 8 Trainium2 cores.

Sharding: core c handles batch b = c//2, query rows [(c%2)*1024, +1024).

Key trick: the key-mask zeroes ~half the KV positions and is known on the
host, so K/V are COMPACTED on the host to the unmasked positions (padded
to a multiple of 128; pad columns get a -1e30 score bias so exp()=0).
This halves scores/exp/attnV work. Each core projects the compacted K/V
for its whole batch itself (cheap), so no collective is needed.

Device layouts (per core):
  xqT (512, 1024), xkT/xvT (512, KVCAP)  inputs, dmodel on partitions
  qT (128, 1024) x4 / kT (128, KVCAP) x4 projections kept transposed:
      head h lives in dmodel-chunk tile h//2 at partition offset 64*(h%2)
  V (128, 520) x KVC   V natural layout per kv chunk; head h at cols
      [65h, 65h+64), col 65h+64 = ones (softmax denominator)
  scores (128kv, 1024q) PSUM; exp+mask+scale fused into one ACT op
  xs (65, 1024) PSUM, row 64 = softmax denominator

Pipeline: per head, chunk c's scores matmuls are emitted one chunk ahead
of chunk c-1's attnV matmuls so the PE never waits on the exp (ACT).
"""
import numpy as np
import ml_dtypes

import concourse.bacc as bacc
import concourse.bass as bass
import concourse.mybir as mybir
import concourse.tile as tile
from concourse.bass_utils import run_bass_kernel_spmd

F32 = mybir.dt.float32
BF16 = mybir.dt.bfloat16
AF = mybir.ActivationFunctionType

B, L, D = 4, 2048, 512
H, DK = 8, 64
N_CORES = 8
LQ = L // 2            # query rows per core
P = 128
QT = LQ // P           # 8 query tiles of 128
MC = D // P            # 4 dmodel chunks
MASK_BIAS = np.float32(-1e30)

MM_DT = BF16
MM_NP = ml_dtypes.bfloat16

_cache = {}


def _plan(mask):
    """KV chunk count after host-side compaction (multiple-of-128 pad)."""
    counts = np.asarray(mask).astype(bool).sum(axis=1)
    kvc = int(np.ceil((counts.max() + 1e-9) / P))
    return max(kvc, 2)


def _build(kvc):
    kvcap = kvc * P
    nc = bacc.Bacc("TRN2", target_bir_lowering=False, debug=False,
                   num_devices=N_CORES)

    xqT_d = nc.dram_tensor("xqT", [D, LQ], MM_DT, kind="ExternalInput").ap()
    xkT_d = nc.dram_tensor("xkT", [D, kvcap], MM_DT, kind="ExternalInput").ap()
    xvT_d = nc.dram_tensor("xvT", [D, kvcap], MM_DT, kind="ExternalInput").ap()
    wq_d = nc.dram_tensor("wq", [D, D], MM_DT, kind="ExternalInput").ap()
    wk_d = nc.dram_tensor("wk", [D, D], MM_DT, kind="ExternalInput").ap()
    wv_d = nc.dram_tensor("wv", [D, D], MM_DT, kind="ExternalInput").ap()
    wo_d = nc.dram_tensor("wo", [D, D], MM_DT, kind="ExternalInput").ap()
    bq_d = nc.dram_tensor("bq", [P, MC], F32, kind="ExternalInput").ap()
    bk_d = nc.dram_tensor("bk", [P, MC], F32, kind="ExternalInput").ap()
    bv_d = nc.dram_tensor("bv", [1, D], MM_DT, kind="ExternalInput").ap()
    bo2_d = nc.dram_tensor("bo2", [1, 2 * D], F32, kind="ExternalInput").ap()
    mb_d = nc.dram_tensor("mb", [P, kvc], F32, kind="ExternalInput").ap()
    out_d = nc.dram_tensor("out", [LQ, D], F32, kind="ExternalOutput").ap()

    kblk = [512] * (kvcap // 512) + ([kvcap % 512] if kvcap % 512 else [])

    with tile.TileContext(nc) as tc:
        with tc.tile_pool(name="const", bufs=1) as cpool, \
             tc.tile_pool(name="xin", bufs=1) as xpool, \
             tc.tile_pool(name="proj", bufs=1) as prpool, \
             tc.tile_pool(name="attn", bufs=6) as apool, \
             tc.tile_pool(name="norm", bufs=2) as npool, \
             tc.tile_pool(name="outp", bufs=2) as opool, \
             tc.tile_pool(name="ps", bufs=2, space="PSUM") as ps:

            def load_chunks(pool, ap2d, nm):
                out = []
                for kc in range(MC):
                    t = pool.tile([P, ap2d.shape[1]], ap2d.dtype,
                                  tag=f"{nm}{kc}", name=f"{nm}{kc}")
                    nc.sync.dma_start(t[:], ap2d[kc * P:(kc + 1) * P, :])
                    out.append(t)
                return out

            # interleave weight/input chunk loads in first-use order
            wq = load_chunks(cpool, wq_d, "wq")
            xqT = load_chunks(xpool, xqT_d, "xq")
            bq = cpool.tile_from(bq_d)
            wk = load_chunks(cpool, wk_d, "wk")
            xkT = load_chunks(xpool, xkT_d, "xk")
            bk = cpool.tile_from(bk_d)
            wv = load_chunks(cpool, wv_d, "wv")
            xvT = load_chunks(xpool, xvT_d, "xv")
            bv = cpool.tile_from(bv_d)
            mb = cpool.tile_from(mb_d)
            wo = load_chunks(cpool, wo_d, "wo")
            bo2 = cpool.tile_from(bo2_d)
            ones1 = cpool.tile([1, P], MM_DT)
            nc.vector.memset(ones1[:], 1.0)
            bo_bc = cpool.tile([P, 2 * D], F32)
            nc.gpsimd.partition_broadcast(bo_bc[:], bo2[:])

            # ---- Q projection (transposed layout), bias-add on ACT ----
            qT = [prpool.tile([P, LQ], MM_DT, tag=f"qT{m}", name=f"qT{m}")
                  for m in range(MC)]
            for m in range(MC):
                pp = ps.tile([P, LQ], F32, tag="sc")
                for s in range(LQ // 512):
                    for kc in range(MC):
                        nc.tensor.matmul(
                            pp[:, s * 512:(s + 1) * 512],
                            wq[kc][:, m * P:(m + 1) * P],
                            xqT[kc][:, s * 512:(s + 1) * 512],
                            start=kc == 0, stop=kc == MC - 1)
                nc.scalar.activation(qT[m][:], pp[:], AF.Identity,
                                     bias=bq[:, m:m + 1])

            # ---- K projection over compacted tokens, bias-add on DVE ----
            kT = [prpool.tile([P, kvcap], MM_DT, tag=f"kT{m}", name=f"kT{m}")
                  for m in range(MC)]
            for m in range(MC):
                off = 0
                for blk in kblk:
                    pk = ps.tile([P, LQ], F32, tag="xs", name="pk")
                    for kc in range(MC):
                        nc.tensor.matmul(
                            pk[:, 0:blk],
                            wk[kc][:, m * P:(m + 1) * P],
                            xkT[kc][:, off:off + blk],
                            start=kc == 0, stop=kc == MC - 1)
                    nc.vector.tensor_scalar_add(kT[m][:, off:off + blk],
                                                pk[:, 0:blk], bk[:, m:m + 1])
                    off += blk

            # ---- V projection (natural layout + ones columns) ----
            V = [prpool.tile([P, H * 65], MM_DT, tag=f"V{t}", name=f"V{t}")
                 for t in range(kvc)]
            for t in range(kvc):
                pv = ps.tile([P, LQ], F32, tag="sc" if t % 2 == 0 else "xs",
                             name="pv")
                for kc in range(MC):
                    nc.tensor.matmul(pv[:, 0:D],
                                     xvT[kc][:, t * P:(t + 1) * P],
                                     wv[kc][:, :], start=kc == 0, stop=False)
                nc.tensor.matmul(pv[:, 0:D], ones1[0:1, :], bv[0:1, :],
                                 start=False, stop=True)
                vv = V[t].rearrange("p (g d) -> p g d", d=65)
                nc.vector.tensor_copy(vv[:, :, 0:64],
                                      pv[:, 0:D].rearrange("p (g d) -> p g d",
                                                           d=64))
                nc.vector.memset(vv[:, :, 64:65], 1.0)

            # ---- flash attention per head, chunk-pipelined ----
            xsT2 = [prpool.tile([P, LQ], MM_DT, tag=f"xs{hp}", name=f"xsT2_{hp}")
                    for hp in range(MC)]

            def scores_chunk(h, c):
                hp, po = h // 2, 64 * (h % 2)
                ss = ps.tile([P, LQ], F32, tag="sc", name=f"ss_h{h}_{c}")
                for qh in range(2):
                    nc.tensor.matmul(
                        ss[:, qh * 512:(qh + 1) * 512],
                        kT[hp][po:po + 64, c * P:(c + 1) * P],
                        qT[hp][po:po + 64, qh * 512:(qh + 1) * 512],
                        start=True, stop=True)
                a = apool.tile([P, LQ], MM_DT, tag="at", name=f"at_h{h}_{c}")
                nc.scalar.activation(a[:], ss[:], AF.Exp,
                                     bias=mb[:, c:c + 1], scale=0.125)
                return a

            def attnv_chunk(h, c, xs, a):
                for qh in range(2):
                    nc.tensor.matmul(
                        xs[:, qh * 512:(qh + 1) * 512],
                        V[c][:, 65 * h:65 * h + 65],
                        a[:, qh * 512:(qh + 1) * 512],
                        start=c == 0, stop=c == kvc - 1)

            for h in range(H):
                hp, po = h // 2, 64 * (h % 2)
                xs = ps.tile([65, LQ], F32, tag="xs", name=f"xs_h{h}")
                at_prev = None
                for c in range(kvc):
                    a = scores_chunk(h, c)
                    if at_prev is not None:
                        attnv_chunk(h, c - 1, xs, at_prev)
                    at_prev = a
                attnv_chunk(h, kvc - 1, xs, at_prev)
                # normalize: row 64 holds the softmax denominator
                rec = npool.tile([1, LQ], F32, tag="rec")
                nc.vector.reciprocal_approx_fast(rec[:], xs[64:65, :])
                bc = npool.tile([64, LQ], F32, tag="bc")
                nc.gpsimd.partition_broadcast(bc[:], rec[:])
                nc.vector.tensor_mul(xsT2[hp][po:po + 64, :],
                                     xs[0:64, :], bc[:])

            # ---- output projection ----
            for q2 in range(QT // 2):
                po_ = ps.tile([P, LQ], F32, tag="sc" if q2 % 2 == 0 else "xs",
                              name="po")
                for sub in range(2):
                    qt = 2 * q2 + sub
                    for hp in range(MC):
                        nc.tensor.matmul(po_[:, sub * 512:(sub + 1) * 512],
                                         xsT2[hp][:, qt * P:(qt + 1) * P],
                                         wo[hp][:, :],
                                         start=hp == 0, stop=hp == MC - 1)
                osb = opool.tile([P, 2 * D], F32, tag="osb")
                nc.vector.tensor_add(osb[:], po_[:], bo_bc[:])
                for sub in range(2):
                    qt = 2 * q2 + sub
                    nc.sync.dma_start(out_d[qt * P:(qt + 1) * P, :],
                                      osb[:, sub * 512:(sub + 1) * 512])

    nc.compile()
    return nc


def _host_inputs(query, key, value, mask, Wq, bq, Wk, bk, Wv, bv, Wo, bo,
                 kvc=None):
    """Build the 8 per-core input maps (all rank-dependence lives here)."""
    f32 = np.float32
    if kvc is None:
        kvc = _plan(mask)
    kvcap = kvc * P
    wq_ = np.ascontiguousarray(Wq).astype(MM_NP)
    wk_ = np.ascontiguousarray(Wk).astype(MM_NP)
    wv_ = np.ascontiguousarray(Wv).astype(MM_NP)
    wo_ = np.ascontiguousarray(Wo).astype(MM_NP)
    bq_ = np.ascontiguousarray(bq.astype(f32).reshape(MC, P).T)
    bk_ = np.ascontiguousarray(bk.astype(f32).reshape(MC, P).T)
    bv_ = bv.astype(MM_NP).reshape(1, D)
    bo2_ = np.tile(bo.astype(f32), 2).reshape(1, 2 * D)
    in_maps = []
    per_batch = {}
    for b in range(B):
        idx = np.flatnonzero(np.asarray(mask[b]) != 0)
        n = len(idx)
        xk = np.zeros((kvcap, D), f32)
        xv = np.zeros((kvcap, D), f32)
        xk[:n] = np.asarray(key[b], f32)[idx]
        xv[:n] = np.asarray(value[b], f32)[idx]
        mbias = np.full(kvcap, MASK_BIAS, f32)
        mbias[:n] = 0.0
        per_batch[b] = (
            np.ascontiguousarray(xk.T).astype(MM_NP),
            np.ascontiguousarray(xv.T).astype(MM_NP),
            np.ascontiguousarray(mbias.reshape(kvc, P).T),
        )
    for c in range(N_CORES):
        b, half = c // 2, c % 2
        sl = slice(half * LQ, (half + 1) * LQ)
        xqT = np.ascontiguousarray(np.asarray(query[b], f32)[sl].T).astype(MM_NP)
        xkT_, xvT_, mb_ = per_batch[b]
        in_maps.append({
            "xqT": xqT, "xkT": xkT_, "xvT": xvT_,
            "wq": wq_, "wk": wk_, "wv": wv_, "wo": wo_,
            "bq": bq_, "bk": bk_, "bv": bv_, "bo2": bo2_, "mb": mb_,
        })
    return in_maps


def kernel(query, key, value, mask, Wq, bq, Wk, bk, Wv, bv, Wo, bo):
    kvc = _plan(mask)
    if kvc not in _cache:
        _cache[kvc] = _build(kvc)
    _cache["nc"] = _cache[kvc]
    nc = _cache[kvc]
    in_maps = _host_inputs(query, key, value, mask,
                           Wq, bq, Wk, bk, Wv, bv, Wo, bo, kvc=kvc)
    res = run_bass_kernel_spmd(nc, in_maps, list(range(N_CORES))).results
    out = np.empty((B, L, D), np.float32)
    for c in range(N_CORES):
        b, half = c // 2, c % 2
        out[b, half * LQ:(half + 1) * LQ, :] = res[c]["out"]
    return out
